# revision 26
# baseline (speedup 1.0000x reference)
"""Trainium2 Bass kernel for a 2-layer GCN encoder (PyG GCNConv semantics).

Strategy (8 NeuronCores, SPMD):
  * Nodes sharded across 8 cores (6250/core); edges partitioned by
    destination shard; weights replicated.
  * Dense layer projections on the local shard; h-tables exchanged with
    AllGather; per-destination-shard aggregation via batched gather DMAs
    (dma_gather, <=1024 rows/call, round-robin over 4 SWDGE queues with
    deep tile-pool buffering to pipeline the DMA descriptor stream) +
    one-hot selection-matrix matmuls on the TensorEngine.
  * The global h tables are split in TWO tile-range classes (local
    tiles [0,28) and [28,49) of every core): each sub-table has
    <=32768 rows so plain int16 gather indices address it, and each is
    AllGathered separately as soon as its producer tiles are done, so
    the collectives pipeline behind compute.
  * Layer-1 tables in fp8e4 (halves AllGather + gather bytes); layer-2
    in fp16. Accumulation in fp32 PSUM.
  * The symmetric norm dinv[src]*dinv[dst] is factored: dinv[src] is
    pre-scaled into the tables (h' = D^-1/2 h), dinv[dst] is a
    per-partition post-scale on the aggregated PSUM. The selection
    matrix is a PURE one-hot (fp8), built per class slice with DVE
    is_equal ops using broadcast access patterns.
  * Self-loops are NOT gathered: the local shard's h' tiles stay
    resident in SBUF and are added via an identity matmul inside the
    PSUM accumulation group. Biases fold in as rank-1 matmuls
    (sqrt(deg)[n] x b[f]), so out = dinv * (R + selfloop + sqd*b).

All preprocessing (degrees, edge bucketing/padding, int16 gather index
tables) happens on the host in numpy inside kernel().
"""

import contextlib
import os
import sys

import numpy as np

for _p in ("/opt/trn_rl_repo", "/root/.axon_site/_ro/trn_rl_repo"):
    if os.path.isdir(_p) and _p not in sys.path:
        sys.path.insert(0, _p)

import concourse.bacc as bacc
import concourse.mybir as mybir
import concourse.tile as tile
from concourse.bass_utils import run_bass_kernel_spmd
from concourse.library_config import mlp as _mlp_lib

P = 128
CORES = 8
GRP = 4  # destination-node tiles per gather group
GCAP = 1024  # max rows per dma_gather call (single-packet ceiling)
NSWQ = 4  # SWDGE queues; gather calls round-robin across them
TBOUNDS = (0, 28)  # class k covers local tiles [TBOUNDS[k], next)

F16 = mybir.dt.float16
F8 = mybir.dt.float8e4
F32 = mybir.dt.float32
I16 = mybir.dt.int16


def _cdiv(a, b):
    return -(-a // b)


class Plan:
    """Static (cross-core identical) schedule + per-core host arrays."""

    def __init__(self, n_nodes, edge_src, edge_dst, in_c, hid, out_c):
        assert n_nodes % CORES == 0
        self.n = n_nodes
        self.inc, self.hid, self.outc = in_c, hid, out_c
        self.shard = n_nodes // CORES
        self.tiles = _cdiv(self.shard, P)
        self.shard_pad = self.tiles * P
        self.npad = self.shard_pad * CORES
        bounds = list(TBOUNDS) + [self.tiles]
        self.ncls = len(TBOUNDS)
        self.cls_lo = bounds[:-1]
        self.cls_w = [bounds[k + 1] - bounds[k] for k in range(self.ncls)]
        self.cls_rows = [CORES * w * P for w in self.cls_w]
        assert all(r <= 32768 for r in self.cls_rows)
        for lo in self.cls_lo[1:]:
            assert lo % GRP == 0  # AG trigger points align with groups

        deg = np.bincount(edge_dst, minlength=n_nodes).astype(np.float64) + 1.0
        self.dinv_full = 1.0 / np.sqrt(deg)
        dinv = self.dinv_full

        shard = self.shard
        core_of = edge_dst // shard
        cnt = np.zeros((self.ncls, CORES, self.tiles), np.int64)
        percore = []
        for c in range(CORES):
            m = core_of == c
            # self-loops are NOT gathered; they are added from the SBUF-
            # resident local h' tiles via an identity matmul.
            s = edge_src[m]
            d = edge_dst[m]
            sc = s // shard  # source core
            sl = s % shard
            st = sl // P  # source local tile
            sp = sl % P
            kls = np.searchsorted(bounds, st, side="right") - 1
            gid = np.zeros_like(s)
            for k in range(self.ncls):
                mk = kls == k
                gid[mk] = (
                    sc[mk] * self.cls_w[k] + (st[mk] - self.cls_lo[k])
                ) * P + sp[mk]
            dloc = d - c * shard
            t = dloc // P
            drel = dloc % P
            percore.append((gid, t, drel, kls))
            for k in range(self.ncls):
                cnt[k, c] = np.bincount(t[kls == k], minlength=self.tiles)
        # chunks per tile per class (max over cores, ceil to 128)
        self.cls_ch = (cnt.max(axis=1) + P - 1) // P  # [ncls, tiles]

        # group schedule
        self.groups = []
        chunk_base = 0
        idxcol_base = 0
        for g0 in range(0, self.tiles, GRP):
            gt = list(range(g0, min(g0 + GRP, self.tiles)))
            off = 0
            tile_chunks = {t: [] for t in gt}
            cls_span = []
            for k in range(self.ncls):
                k0 = off
                for t in gt:
                    tile_chunks[t].append((off, int(self.cls_ch[k, t])))
                    off += int(self.cls_ch[k, t])
                cls_span.append((k0, off - k0))
            self.groups.append(
                dict(
                    tiles=gt,
                    cls_span=cls_span,
                    nch=off,
                    chunk_base=chunk_base,
                    idxcol_base=idxcol_base,
                    tile_chunks=tile_chunks,
                )
            )
            chunk_base += off
            idxcol_base += off * P // 16
        self.tot_chunks = chunk_base
        self.tot_idxcols = idxcol_base

        # per-core flat edge-position arrays in canonical (call-major) order
        self.core_idx = []
        self.core_drel = []
        self.core_dinvc = []
        self.core_sqd = []
        for c in range(CORES):
            gid, t_arr, drel, kls = percore[c]
            idx_flat = np.zeros(self.tot_chunks * P, np.int32)
            drel_flat = np.full(self.tot_chunks * P, -1.0, np.float32)
            for g in self.groups:
                pos = g["chunk_base"] * P
                for k in range(self.ncls):
                    for t in g["tiles"]:
                        sel = (t_arr == t) & (kls == k)
                        kk = int(sel.sum())
                        idx_flat[pos : pos + kk] = gid[sel]
                        drel_flat[pos : pos + kk] = drel[sel]
                        pos += int(self.cls_ch[k, t]) * P
            # wrap idx into the dma_gather SBUF layout [128, cols]: per
            # call, idx i lives at [p, i // 16] for p % 16 == i % 16.
            # Calls are <=GCAP rows, so wrap per GCAP-sized piece.
            blocks = []
            for g in self.groups:
                a = g["chunk_base"] * P
                for k0, kn in g["cls_span"]:
                    n = kn * P
                    for off in range(0, n, GCAP):
                        nn = min(GCAP, n - off)
                        v = idx_flat[a + off : a + off + nn].reshape(-1, 16).T
                        blocks.append(np.tile(v, (8, 1)))
                    a += n
            idx_sb = (
                np.concatenate(blocks, axis=1).astype(np.int16)
                if blocks
                else np.zeros((P, 0), np.int16)
            )
            assert idx_sb.shape == (P, self.tot_idxcols)
            self.core_idx.append(np.ascontiguousarray(idx_sb))
            self.core_drel.append(
                np.ascontiguousarray(
                    drel_flat.reshape(self.tot_chunks, P).T.astype(np.float32)
                )
            )
            # per-node scale planes for this shard (pad nodes: dinv=1, sqd=0)
            dshard = np.ones(self.shard_pad, np.float64)
            dshard[:shard] = dinv[c * shard : (c + 1) * shard]
            dc = dshard.reshape(self.tiles, P).T.astype(np.float32)  # [P, tiles]
            self.core_dinvc.append(np.ascontiguousarray(dc))
            sq = np.zeros(self.shard_pad, np.float64)
            sq[:shard] = 1.0 / dinv[c * shard : (c + 1) * shard]
            self.core_sqd.append(sq.astype(np.float16).reshape(1, self.shard_pad))


def _build_nc(plan):
    inc, hid, outc = plan.inc, plan.hid, plan.outc
    ncc = inc // P  # contraction chunks for layer-1 projection
    hcc = hid // P  # contraction chunks for layer-2 projection
    tiles = plan.tiles
    ncls = plan.ncls

    nc = bacc.Bacc("TRN2", num_devices=CORES, num_swdge_queues=NSWQ)

    xt_d = nc.dram_tensor("xt", [tiles, P, ncc, P], F16, kind="ExternalInput")
    w1_d = nc.dram_tensor("w1t", [P, ncc, hid], F16, kind="ExternalInput")
    w2_d = nc.dram_tensor("w2t", [P, hcc, outc], F16, kind="ExternalInput")
    b1_d = nc.dram_tensor("b1r", [1, hid], F16, kind="ExternalInput")
    b2_d = nc.dram_tensor("b2r", [1, outc], F16, kind="ExternalInput")
    io_d = nc.dram_tensor("iota", [P, P], F32, kind="ExternalInput")
    id_d = nc.dram_tensor("ident", [P, P], F16, kind="ExternalInput")
    ix_d = nc.dram_tensor("idx", [P, plan.tot_idxcols], I16, kind="ExternalInput")
    dr_d = nc.dram_tensor("dstrel", [P, plan.tot_chunks], F32, kind="ExternalInput")
    dv_d = nc.dram_tensor("dinvc", [P, tiles], F32, kind="ExternalInput")
    sq_d = nc.dram_tensor("sqd", [1, plan.shard_pad], F16, kind="ExternalInput")
    out_d = nc.dram_tensor("out", [plan.shard_pad, outc], F32, kind="ExternalOutput")

    h1_sh = nc.dram_tensor("h1_shard", [plan.shard_pad, hid], F8)
    h2_sh = nc.dram_tensor("h2_shard", [plan.shard_pad, outc], F16)
    h1_f = [
        nc.dram_tensor(f"h1_f{k}", [plan.cls_rows[k], hid], F8, addr_space="Shared")
        for k in range(ncls)
    ]
    h2_f = [
        nc.dram_tensor(f"h2_f{k}", [plan.cls_rows[k], outc], F16, addr_space="Shared")
        for k in range(ncls)
    ]

    st = contextlib.ExitStack()
    h1loc = st.enter_context(nc.sbuf_tensor("h1loc", [P, tiles, hid], F8))
    h2loc = st.enter_context(nc.sbuf_tensor("h2loc", [P, tiles, outc], F16))
    idx_sb = st.enter_context(nc.sbuf_tensor("idx_sb", [P, plan.tot_idxcols], I16))
    dr_sb = st.enter_context(nc.sbuf_tensor("dr_sb", [P, plan.tot_chunks], F32))
    io_sb = st.enter_context(nc.sbuf_tensor("io_sb", [P, P], F32))
    id_sb = st.enter_context(nc.sbuf_tensor("id_sb", [P, P], F16))
    w1_sb = st.enter_context(nc.sbuf_tensor("w1_sb", [P, ncc, hid], F16))
    w2_sb = st.enter_context(nc.sbuf_tensor("w2_sb", [P, hcc, outc], F16))
    b1_sb = st.enter_context(nc.sbuf_tensor("b1_sb", [1, hid], F16))
    b2_sb = st.enter_context(nc.sbuf_tensor("b2_sb", [1, outc], F16))
    dv_sb = st.enter_context(nc.sbuf_tensor("dv_sb", [P, tiles], F32))
    sq_sb = st.enter_context(nc.sbuf_tensor("sq_sb", [1, plan.shard_pad], F16))

    nc.gpsimd.load_library(_mlp_lib)

    # one gpsimd register per distinct gather count
    _regs = {}

    def _nreg(v):
        if v not in _regs:
            _regs[v] = nc.gpsimd.to_reg(v)
        return _regs[v]

    for g in plan.groups:
        for _, kn in g["cls_span"]:
            n = kn * P
            for off in range(0, n, GCAP):
                _nreg(min(GCAP, n - off))

    _qrr = [0]  # round-robin SWDGE queue counter

    def _emit_gather(msg, ch0, nch, table, ic0, elem):
        """Gather nch*P rows into msg[:, ch0:ch0+nch, :] in <=GCAP pieces."""
        n = nch * P
        ic = ic0
        for off in range(0, n, GCAP):
            nn = min(GCAP, n - off)
            nc.gpsimd.dma_gather(
                msg[:, ch0 + off // P : ch0 + (off + nn) // P, :],
                table,
                idx_sb[:, ic : ic + nn // 16],
                nn,
                _nreg(nn),
                elem,
                queue_num=_qrr[0] % NSWQ,
            )
            _qrr[0] += 1
            ic += nn // 16
        return ic

    def _allgather(src_ap, dst_ap):
        return nc.gpsimd.collective_compute(
            "AllGather",
            mybir.AluOpType.bypass,
            replica_groups=[list(range(CORES))],
            ins=[src_ap],
            outs=[dst_ap],
        )

    def _emit_group_gathers(g, msg, tabs, elem):
        ic = g["idxcol_base"]
        for k in range(ncls):
            k0, kn = g["cls_span"][k]
            if kn:
                ic = _emit_gather(msg, k0, kn, tabs[k][:, :], ic, elem)

    # ctx0: resident loads (own context so its exit barrier orders them
    # before every later consumer on every engine).
    with tile.TileContext(nc):
        nc.sync.dma_start(idx_sb[:, :], ix_d[:, :])
        nc.sync.dma_start(dr_sb[:, :], dr_d[:, :])
        nc.sync.dma_start(io_sb[:, :], io_d[:, :])
        nc.sync.dma_start(id_sb[:, :], id_d[:, :])
        nc.sync.dma_start(w1_sb[:, :, :], w1_d[:, :, :])
        nc.sync.dma_start(w2_sb[:, :, :], w2_d[:, :, :])
        nc.sync.dma_start(b1_sb[:, :], b1_d[:, :])
        nc.sync.dma_start(b2_sb[:, :], b2_d[:, :])
        nc.sync.dma_start(dv_sb[:, :], dv_d[:, :])
        nc.sync.dma_start(sq_sb[:, :], sq_d[:, :])

    MEGA = tile.TileContext(nc)
    tc_main = MEGA.__enter__()

    # layer-1 dense projection h1' = D^-1/2 (x @ W1), 2 tiles/step; the
    # class-k AllGather fires as soon as its producer tiles are written.
    def _dense1(tc, t_lo, t_hi):
        with (
            tc.tile_pool(name=f"pA{t_lo}", bufs=3) as pA,
            tc.tile_pool(name=f"psA{t_lo}", bufs=4, space="PSUM") as psA,
        ):
            for t0 in range(t_lo, t_hi, 2):
                nt = min(2, t_hi - t0)
                xa = pA.tile([P, nt, ncc, P], F16, tag="xa")
                nc.sync.dma_start(
                    xa[:, :, :, :],
                    xt_d[t0 : t0 + nt, :, :, :].rearrange("t p c n -> p t c n"),
                )
                for kk in range(nt):
                    ps = psA.tile([P, hid], F32, tag="psA")
                    for cc in range(ncc):
                        nc.tensor.matmul(
                            ps[:, :],
                            lhsT=xa[:, kk, cc, :],
                            rhs=w1_sb[:, cc, :],
                            start=(cc == 0),
                            stop=(cc == ncc - 1),
                        )
                    nc.scalar.activation(
                        h1loc[:, t0 + kk, :],
                        ps[:, :],
                        mybir.ActivationFunctionType.Copy,
                        scale=dv_sb[:, t0 + kk : t0 + kk + 1],
                    )
                nc.sync.dma_start(
                    h1_sh[t0 * P : (t0 + nt) * P, :].rearrange(
                        "(t p) f -> p t f", p=P
                    ),
                    h1loc[:, t0 : t0 + nt, :],
                )

    cls_hi = list(plan.cls_lo[1:]) + [tiles]
    for k in range(ncls):
        _dense1(tc_main, plan.cls_lo[k], cls_hi[k])
        _allgather(h1_sh[plan.cls_lo[k] * P : cls_hi[k] * P, :], h1_f[k][:, :])

    # layer-1 aggregate + relu + layer-2 dense projection, group by group;
    # the class-k h2 AllGather fires once its tiles are aggregated.
    def _agg1(tc, g_lo, g_hi):
        groups = plan.groups[g_lo:g_hi]
        with (
            tc.tile_pool(name=f"msgB{g_lo}", bufs=3) as msgB,
            tc.tile_pool(name=f"sB{g_lo}", bufs=3) as sB,
            tc.tile_pool(name=f"zB{g_lo}", bufs=3) as zB,
            tc.tile_pool(name=f"psB{g_lo}", bufs=4, space="PSUM") as psB,
            tc.tile_pool(name=f"psT{g_lo}", bufs=2, space="PSUM") as psT,
            tc.tile_pool(name=f"psH{g_lo}", bufs=2, space="PSUM") as psH,
        ):
            def ega(g, m):
                k0, kn = g["cls_span"][0]
                if kn:
                    _emit_gather(m, k0, kn, h1_f[0][:, :], g["idxcol_base"], hid)

            def egb(g, m):
                k0, kn = g["cls_span"][1]
                if kn:
                    _emit_gather(
                        m, k0, kn, h1_f[1][:, :],
                        g["idxcol_base"] + g["cls_span"][0][1] * P // 16, hid,
                    )

            msgs = {}
            for i in range(min(2, len(groups))):
                msgs[i] = msgB.tile(
                    [P, groups[i]["nch"], hid], F8, tag="msg", name=f"mg1_{g_lo}_{i}"
                )
                ega(groups[i], msgs[i])
            for i, g in enumerate(groups):
                egb(g, msgs[i])
                if i + 2 < len(groups):
                    msgs[i + 2] = msgB.tile(
                        [P, groups[i + 2]["nch"], hid], F8,
                        tag="msg", name=f"mg1_{g_lo}_{i+2}",
                    )
                    ega(groups[i + 2], msgs[i + 2])
                msg = msgs.pop(i)
                for t in g["tiles"]:
                    spans = g["tile_chunks"][t]
                    nch = sum(kn for _, kn in spans)
                    # one-hot S, one DVE is_equal per class slice
                    S = sB.tile([P, nch, P], F8, tag="S")
                    soff = 0
                    for k0, kn in spans:
                        if not kn:
                            continue
                        nc.vector.tensor_tensor(
                            out=S[:, soff : soff + kn, :],
                            in0=io_sb[:, None, :].to_broadcast([P, kn, P]),
                            in1=dr_sb[
                                :, g["chunk_base"] + k0 : g["chunk_base"] + k0 + kn
                            ][:, :, None].to_broadcast([P, kn, P]),
                            op=mybir.AluOpType.is_equal,
                        )
                        soff += kn
                    ps = psB.tile([P, hid], F32, tag="psB")
                    # rank-1 bias + self-loop first: independent of gathers
                    nc.tensor.matmul(
                        ps[:, :],
                        lhsT=sq_sb[0:1, t * P : (t + 1) * P],
                        rhs=b1_sb[0:1, :],
                        start=True,
                        stop=False,
                    )
                    nc.tensor.matmul(
                        ps[:, :],
                        lhsT=id_sb[:, :],
                        rhs=h1loc[:, t, :],
                        start=False,
                        stop=(nch == 0),
                    )
                    j = 0
                    for k0, kn in spans:
                        for i in range(kn):
                            nc.tensor.matmul(
                                ps[:, :],
                                lhsT=S[:, j, :],
                                rhs=msg[:, k0 + i, :],
                                start=False,
                                stop=(j == nch - 1),
                            )
                            j += 1
                    # z1 = relu(dinv[n] * ps)  [node, f] fp16
                    z1 = zB.tile([P, hid], F16, tag="z1")
                    nc.vector.tensor_scalar(
                        out=z1[:, :],
                        in0=ps[:, :],
                        scalar1=dv_sb[:, t : t + 1],
                        scalar2=0.0,
                        op0=mybir.AluOpType.mult,
                        op1=mybir.AluOpType.max,
                    )
                    # transpose z1 -> z1T for the layer-2 contraction
                    zt_ps = psT.tile([P, hcc, P], F16, tag="ztps")
                    for h in range(hcc):
                        nc.tensor.transpose(
                            zt_ps[:, h, :],
                            z1[:, h * P : (h + 1) * P],
                            id_sb[:, :],
                        )
                    zt = zB.tile([P, hcc, P], F16, tag="zt")
                    nc.scalar.activation(
                        zt[:, :, :],
                        zt_ps[:, :, :],
                        mybir.ActivationFunctionType.Copy,
                    )
                    hps = psH.tile([P, outc], F32, tag="hps")
                    for cc in range(hcc):
                        nc.tensor.matmul(
                            hps[:, :],
                            lhsT=zt[:, cc, :],
                            rhs=w2_sb[:, cc, :],
                            start=(cc == 0),
                            stop=(cc == hcc - 1),
                        )
                    # h2' = dinv[n] * (z1 @ W2)
                    nc.scalar.activation(
                        h2loc[:, t, :],
                        hps[:, :],
                        mybir.ActivationFunctionType.Copy,
                        scale=dv_sb[:, t : t + 1],
                    )
                    nc.sync.dma_start(
                        h2_sh[t * P : (t + 1) * P, :], h2loc[:, t, :]
                    )

    for k in range(ncls):
        _agg1(tc_main, plan.cls_lo[k] // GRP, _cdiv(cls_hi[k], GRP))
        _allgather(h2_sh[plan.cls_lo[k] * P : cls_hi[k] * P, :], h2_f[k][:, :])

    # layer-2 aggregate + bias -> output
    with (
        tc_main.tile_pool(name="msgC", bufs=3) as msgC,
        tc_main.tile_pool(name="sC", bufs=3) as sC,
        tc_main.tile_pool(name="oC", bufs=3) as oC,
        tc_main.tile_pool(name="psC", bufs=6, space="PSUM") as psC,
    ):
        def ega2(g, m):
            k0, kn = g["cls_span"][0]
            if kn:
                _emit_gather(m, k0, kn, h2_f[0][:, :], g["idxcol_base"], outc)

        def egb2(g, m):
            k0, kn = g["cls_span"][1]
            if kn:
                _emit_gather(
                    m, k0, kn, h2_f[1][:, :],
                    g["idxcol_base"] + g["cls_span"][0][1] * P // 16, outc,
                )

        msgs2 = {}
        for i in range(min(2, len(plan.groups))):
            msgs2[i] = msgC.tile(
                [P, plan.groups[i]["nch"], outc], F16, tag="msg2", name=f"mg2_{i}"
            )
            ega2(plan.groups[i], msgs2[i])
        for i, g in enumerate(plan.groups):
            egb2(g, msgs2[i])
            if i + 2 < len(plan.groups):
                msgs2[i + 2] = msgC.tile(
                    [P, plan.groups[i + 2]["nch"], outc], F16,
                    tag="msg2", name=f"mg2_{i+2}",
                )
                ega2(plan.groups[i + 2], msgs2[i + 2])
            msg = msgs2.pop(i)
            for t in g["tiles"]:
                spans = g["tile_chunks"][t]
                nch = sum(kn for _, kn in spans)
                S = sC.tile([P, nch, P], F8, tag="S2")
                soff = 0
                for k0, kn in spans:
                    if not kn:
                        continue
                    nc.vector.tensor_tensor(
                        out=S[:, soff : soff + kn, :],
                        in0=io_sb[:, None, :].to_broadcast([P, kn, P]),
                        in1=dr_sb[
                            :, g["chunk_base"] + k0 : g["chunk_base"] + k0 + kn
                        ][:, :, None].to_broadcast([P, kn, P]),
                        op=mybir.AluOpType.is_equal,
                    )
                    soff += kn
                ps = psC.tile([P, outc], F32, tag="psC")
                nc.tensor.matmul(
                    ps[:, :],
                    lhsT=sq_sb[0:1, t * P : (t + 1) * P],
                    rhs=b2_sb[0:1, :],
                    start=True,
                    stop=False,
                )
                nc.tensor.matmul(
                    ps[:, :],
                    lhsT=id_sb[:, :],
                    rhs=h2loc[:, t, :],
                    start=False,
                    stop=(nch == 0),
                )
                j = 0
                for k0, kn in spans:
                    for i in range(kn):
                        nc.tensor.matmul(
                            ps[:, :],
                            lhsT=S[:, j, :],
                            rhs=msg[:, k0 + i, :],
                            start=False,
                            stop=(j == nch - 1),
                        )
                        j += 1
                ob = oC.tile([P, outc], F32, tag="ob")
                nc.vector.tensor_scalar(
                    out=ob[:, :],
                    in0=ps[:, :],
                    scalar1=dv_sb[:, t : t + 1],
                    scalar2=None,
                    op0=mybir.AluOpType.mult,
                )
                nc.sync.dma_start(out_d[t * P : (t + 1) * P, :], ob[:, :])

    MEGA.__exit__(None, None, None)
    st.close()
    nc.compile()
    return nc


def _make_in_maps(plan, x, W1, b1, W2, b2):
    inc, hid, outc = plan.inc, plan.hid, plan.outc
    ncc, hcc = inc // P, hid // P
    w1t = np.ascontiguousarray(
        W1.reshape(ncc, P, hid).transpose(1, 0, 2).astype(np.float16)
    )
    w2t = np.ascontiguousarray(
        W2.reshape(hcc, P, outc).transpose(1, 0, 2).astype(np.float16)
    )
    b1r = np.ascontiguousarray(b1.astype(np.float16).reshape(1, hid))
    b2r = np.ascontiguousarray(b2.astype(np.float16).reshape(1, outc))
    iota = np.ascontiguousarray(
        np.tile(np.arange(P, dtype=np.float32), (P, 1))
    )
    ident = np.ascontiguousarray(np.eye(P, dtype=np.float16))
    in_maps = []
    for c in range(CORES):
        xs = x[c * plan.shard : (c + 1) * plan.shard].astype(np.float32)
        xs = np.pad(xs, ((0, plan.shard_pad - plan.shard), (0, 0)))
        xt = xs.reshape(plan.tiles, P, ncc, P).transpose(0, 3, 2, 1)
        in_maps.append(
            {
                "xt": np.ascontiguousarray(xt.astype(np.float16)),
                "w1t": w1t,
                "w2t": w2t,
                "b1r": b1r,
                "b2r": b2r,
                "iota": iota,
                "ident": ident,
                "idx": plan.core_idx[c],
                "dstrel": plan.core_drel[c],
                "dinvc": plan.core_dinvc[c],
                "sqd": plan.core_sqd[c],
            }
        )
    return in_maps


_CACHE = {}


def _get_built(x, edge_index, W1, b1, W2, b2):
    n_nodes, in_c = x.shape
    hid = W1.shape[1]
    out_c = W2.shape[1]
    key = (n_nodes, in_c, hid, out_c, hash(edge_index.tobytes()))
    if key not in _CACHE:
        src = np.asarray(edge_index[0], np.int64)
        dst = np.asarray(edge_index[1], np.int64)
        plan = Plan(n_nodes, src, dst, in_c, hid, out_c)
        nc = _build_nc(plan)
        _CACHE[key] = (plan, nc)
    return _CACHE[key]


def run(x, edge_index, W1, b1, W2, b2, trace=False, **spmd_kwargs):
    plan, nc = _get_built(x, edge_index, W1, b1, W2, b2)
    in_maps = _make_in_maps(plan, x, W1, b1, W2, b2)
    res = run_bass_kernel_spmd(
        nc, in_maps, core_ids=list(range(CORES)), trace=trace, **spmd_kwargs
    )
    out = np.concatenate(
        [res.results[c]["out"][: plan.shard] for c in range(CORES)], axis=0
    ).astype(np.float32)
    return out, res


def kernel(**inputs):
    x = np.asarray(inputs["x"], np.float32)
    edge_index = np.asarray(inputs["edge_index"])
    W1 = np.asarray(inputs["W1"], np.float32)
    b1 = np.asarray(inputs["b1"], np.float32)
    W2 = np.asarray(inputs["W2"], np.float32)
    b2 = np.asarray(inputs["b2"], np.float32)
    out, _ = run(x, edge_index, W1, b1, W2, b2)
    return out


# revision 27
# speedup vs baseline: 1.1004x; 1.1004x over previous
"""Trainium2 Bass kernel for a 2-layer GCN encoder (PyG GCNConv semantics).

Strategy (8 NeuronCores, SPMD):
  * Nodes sharded across 8 cores (6250/core); edges partitioned by
    destination shard; weights replicated.
  * Dense layer projections on the local shard; h-tables exchanged with
    AllGather; per-destination-shard aggregation via batched gather DMAs
    (dma_gather, <=1024 rows/call, round-robin over 4 SWDGE queues with
    deep tile-pool buffering to pipeline the DMA descriptor stream) +
    one-hot selection-matrix matmuls on the TensorEngine.
  * The global h tables are split in TWO tile-range classes (local
    tiles [0,28) and [28,49) of every core): each sub-table has
    <=32768 rows so plain int16 gather indices address it, and each is
    AllGathered separately as soon as its producer tiles are done, so
    the collectives pipeline behind compute.
  * Layer-1 tables in fp8e4 (halves AllGather + gather bytes); layer-2
    in fp16. Accumulation in fp32 PSUM.
  * The symmetric norm dinv[src]*dinv[dst] is factored: dinv[src] is
    pre-scaled into the tables (h' = D^-1/2 h), dinv[dst] is a
    per-partition post-scale on the aggregated PSUM. The selection
    matrix is a PURE one-hot (fp8), built per class slice with DVE
    is_equal ops using broadcast access patterns.
  * Self-loops are NOT gathered: the local shard's h' tiles stay
    resident in SBUF and are added via an identity matmul inside the
    PSUM accumulation group. Biases fold in as rank-1 matmuls
    (sqrt(deg)[n] x b[f]), so out = dinv * (R + selfloop + sqd*b).

All preprocessing (degrees, edge bucketing/padding, int16 gather index
tables) happens on the host in numpy inside kernel().
"""

import contextlib
import os
import sys

import numpy as np

for _p in ("/opt/trn_rl_repo", "/root/.axon_site/_ro/trn_rl_repo"):
    if os.path.isdir(_p) and _p not in sys.path:
        sys.path.insert(0, _p)

import concourse.bacc as bacc
import concourse.mybir as mybir
import concourse.tile as tile
from concourse.bass_utils import run_bass_kernel_spmd
from concourse.library_config import mlp as _mlp_lib

P = 128
CORES = 8
GRP = 4  # destination-node tiles per gather group
GCAP = 1024  # max rows per dma_gather call (single-packet ceiling)
NSWQ = 4  # SWDGE queues; gather calls round-robin across them
TBOUNDS = (0, 28)  # class k covers local tiles [TBOUNDS[k], next)

F16 = mybir.dt.float16
F8 = mybir.dt.float8e4
F32 = mybir.dt.float32
I16 = mybir.dt.int16


def _cdiv(a, b):
    return -(-a // b)


class Plan:
    """Static (cross-core identical) schedule + per-core host arrays."""

    def __init__(self, n_nodes, edge_src, edge_dst, in_c, hid, out_c):
        assert n_nodes % CORES == 0
        self.n = n_nodes
        self.inc, self.hid, self.outc = in_c, hid, out_c
        self.shard = n_nodes // CORES
        self.tiles = _cdiv(self.shard, P)
        self.shard_pad = self.tiles * P
        self.npad = self.shard_pad * CORES
        bounds = list(TBOUNDS) + [self.tiles]
        self.ncls = len(TBOUNDS)
        self.cls_lo = bounds[:-1]
        self.cls_w = [bounds[k + 1] - bounds[k] for k in range(self.ncls)]
        self.cls_rows = [CORES * w * P for w in self.cls_w]
        assert all(r <= 32768 for r in self.cls_rows)
        for lo in self.cls_lo[1:]:
            assert lo % GRP == 0  # AG trigger points align with groups

        deg = np.bincount(edge_dst, minlength=n_nodes).astype(np.float64) + 1.0
        self.dinv_full = 1.0 / np.sqrt(deg)
        dinv = self.dinv_full

        shard = self.shard
        core_of = edge_dst // shard
        cnt = np.zeros((self.ncls, CORES, self.tiles), np.int64)
        percore = []
        for c in range(CORES):
            m = core_of == c
            # self-loops are NOT gathered; they are added from the SBUF-
            # resident local h' tiles via an identity matmul.
            s = edge_src[m]
            d = edge_dst[m]
            sc = s // shard  # source core
            sl = s % shard
            st = sl // P  # source local tile
            sp = sl % P
            kls = np.searchsorted(bounds, st, side="right") - 1
            gid = np.zeros_like(s)
            for k in range(self.ncls):
                mk = kls == k
                gid[mk] = (
                    sc[mk] * self.cls_w[k] + (st[mk] - self.cls_lo[k])
                ) * P + sp[mk]
            dloc = d - c * shard
            t = dloc // P
            drel = dloc % P
            percore.append((gid, t, drel, kls))
            for k in range(self.ncls):
                cnt[k, c] = np.bincount(t[kls == k], minlength=self.tiles)
        # chunks per tile per class (max over cores, ceil to 128)
        self.cls_ch = (cnt.max(axis=1) + P - 1) // P  # [ncls, tiles]

        # group schedule
        self.groups = []
        chunk_base = 0
        idxcol_base = 0
        for g0 in range(0, self.tiles, GRP):
            gt = list(range(g0, min(g0 + GRP, self.tiles)))
            off = 0
            tile_chunks = {t: [] for t in gt}
            cls_span = []
            for k in range(self.ncls):
                k0 = off
                for t in gt:
                    tile_chunks[t].append((off, int(self.cls_ch[k, t])))
                    off += int(self.cls_ch[k, t])
                cls_span.append((k0, off - k0))
            self.groups.append(
                dict(
                    tiles=gt,
                    cls_span=cls_span,
                    nch=off,
                    chunk_base=chunk_base,
                    idxcol_base=idxcol_base,
                    tile_chunks=tile_chunks,
                )
            )
            chunk_base += off
            idxcol_base += off * P // 16
        self.tot_chunks = chunk_base
        self.tot_idxcols = idxcol_base

        # per-core flat edge-position arrays in canonical (call-major) order
        self.core_idx = []
        self.core_drel = []
        self.core_dinvc = []
        self.core_sqd = []
        for c in range(CORES):
            gid, t_arr, drel, kls = percore[c]
            idx_flat = np.zeros(self.tot_chunks * P, np.int32)
            drel_flat = np.full(self.tot_chunks * P, -1.0, np.float32)
            for g in self.groups:
                pos = g["chunk_base"] * P
                for k in range(self.ncls):
                    for t in g["tiles"]:
                        sel = (t_arr == t) & (kls == k)
                        kk = int(sel.sum())
                        idx_flat[pos : pos + kk] = gid[sel]
                        drel_flat[pos : pos + kk] = drel[sel]
                        pos += int(self.cls_ch[k, t]) * P
            # wrap idx into the dma_gather SBUF layout [128, cols]: per
            # call, idx i lives at [p, i // 16] for p % 16 == i % 16.
            # Calls are <=GCAP rows, so wrap per GCAP-sized piece.
            blocks = []
            for g in self.groups:
                a = g["chunk_base"] * P
                for k0, kn in g["cls_span"]:
                    n = kn * P
                    for off in range(0, n, GCAP):
                        nn = min(GCAP, n - off)
                        v = idx_flat[a + off : a + off + nn].reshape(-1, 16).T
                        blocks.append(np.tile(v, (8, 1)))
                    a += n
            idx_sb = (
                np.concatenate(blocks, axis=1).astype(np.int16)
                if blocks
                else np.zeros((P, 0), np.int16)
            )
            assert idx_sb.shape == (P, self.tot_idxcols)
            self.core_idx.append(np.ascontiguousarray(idx_sb))
            self.core_drel.append(
                np.ascontiguousarray(
                    drel_flat.reshape(self.tot_chunks, P).T.astype(np.float32)
                )
            )
            # per-node scale planes for this shard (pad nodes: dinv=1, sqd=0)
            dshard = np.ones(self.shard_pad, np.float64)
            dshard[:shard] = dinv[c * shard : (c + 1) * shard]
            dc = dshard.reshape(self.tiles, P).T.astype(np.float32)  # [P, tiles]
            self.core_dinvc.append(np.ascontiguousarray(dc))
            sq = np.zeros(self.shard_pad, np.float64)
            sq[:shard] = 1.0 / dinv[c * shard : (c + 1) * shard]
            self.core_sqd.append(sq.astype(np.float16).reshape(1, self.shard_pad))


def _build_nc(plan):
    inc, hid, outc = plan.inc, plan.hid, plan.outc
    ncc = inc // P  # contraction chunks for layer-1 projection
    hcc = hid // P  # contraction chunks for layer-2 projection
    tiles = plan.tiles
    ncls = plan.ncls

    nc = bacc.Bacc("TRN2", num_devices=CORES, num_swdge_queues=NSWQ)

    xt_d = nc.dram_tensor("xt", [tiles, P, ncc, P], F16, kind="ExternalInput")
    w1_d = nc.dram_tensor("w1t", [P, ncc, hid], F16, kind="ExternalInput")
    w2_d = nc.dram_tensor("w2t", [P, hcc, outc], F16, kind="ExternalInput")
    b1_d = nc.dram_tensor("b1r", [1, hid], F16, kind="ExternalInput")
    b2_d = nc.dram_tensor("b2r", [1, outc], F16, kind="ExternalInput")
    io_d = nc.dram_tensor("iota", [P, P], F32, kind="ExternalInput")
    id_d = nc.dram_tensor("ident", [P, P], F16, kind="ExternalInput")
    ix_d = nc.dram_tensor("idx", [P, plan.tot_idxcols], I16, kind="ExternalInput")
    dr_d = nc.dram_tensor("dstrel", [P, plan.tot_chunks], F32, kind="ExternalInput")
    dv_d = nc.dram_tensor("dinvc", [P, tiles], F32, kind="ExternalInput")
    sq_d = nc.dram_tensor("sqd", [1, plan.shard_pad], F16, kind="ExternalInput")
    out_d = nc.dram_tensor("out", [plan.shard_pad, outc], F32, kind="ExternalOutput")

    h1_sh = nc.dram_tensor("h1_shard", [plan.shard_pad, hid], F8)
    h2_sh = nc.dram_tensor("h2_shard", [plan.shard_pad, outc], F16)
    h1_f = [
        nc.dram_tensor(f"h1_f{k}", [plan.cls_rows[k], hid], F8, addr_space="Shared")
        for k in range(ncls)
    ]
    h2_f = [
        nc.dram_tensor(f"h2_f{k}", [plan.cls_rows[k], outc], F16, addr_space="Shared")
        for k in range(ncls)
    ]

    st = contextlib.ExitStack()
    h1loc = st.enter_context(nc.sbuf_tensor("h1loc", [P, tiles, hid], F8))
    h2loc = st.enter_context(nc.sbuf_tensor("h2loc", [P, tiles, outc], F16))
    idx_sb = st.enter_context(nc.sbuf_tensor("idx_sb", [P, plan.tot_idxcols], I16))
    dr_sb = st.enter_context(nc.sbuf_tensor("dr_sb", [P, plan.tot_chunks], F32))
    io_sb = st.enter_context(nc.sbuf_tensor("io_sb", [P, P], F32))
    id_sb = st.enter_context(nc.sbuf_tensor("id_sb", [P, P], F16))
    w1_sb = st.enter_context(nc.sbuf_tensor("w1_sb", [P, ncc, hid], F16))
    w2_sb = st.enter_context(nc.sbuf_tensor("w2_sb", [P, hcc, outc], F16))
    b1_sb = st.enter_context(nc.sbuf_tensor("b1_sb", [1, hid], F16))
    b2_sb = st.enter_context(nc.sbuf_tensor("b2_sb", [1, outc], F16))
    dv_sb = st.enter_context(nc.sbuf_tensor("dv_sb", [P, tiles], F32))
    sq_sb = st.enter_context(nc.sbuf_tensor("sq_sb", [1, plan.shard_pad], F16))

    nc.gpsimd.load_library(_mlp_lib)

    # one gpsimd register per distinct gather count
    _regs = {}

    def _nreg(v):
        if v not in _regs:
            _regs[v] = nc.gpsimd.to_reg(v)
        return _regs[v]

    for g in plan.groups:
        for _, kn in g["cls_span"]:
            n = kn * P
            for off in range(0, n, GCAP):
                _nreg(min(GCAP, n - off))

    _qrr = [0]  # round-robin SWDGE queue counter

    def _emit_gather(msg, ch0, nch, table, ic0, elem):
        """Gather nch*P rows into msg[:, ch0:ch0+nch, :] in <=GCAP pieces."""
        n = nch * P
        ic = ic0
        for off in range(0, n, GCAP):
            nn = min(GCAP, n - off)
            nc.gpsimd.dma_gather(
                msg[:, ch0 + off // P : ch0 + (off + nn) // P, :],
                table,
                idx_sb[:, ic : ic + nn // 16],
                nn,
                _nreg(nn),
                elem,
                queue_num=_qrr[0] % NSWQ,
            )
            _qrr[0] += 1
            ic += nn // 16
        return ic

    def _allgather(src_ap, dst_ap):
        return nc.gpsimd.collective_compute(
            "AllGather",
            mybir.AluOpType.bypass,
            replica_groups=[list(range(CORES))],
            ins=[src_ap],
            outs=[dst_ap],
        )

    def _emit_group_gathers(g, msg, tabs, elem):
        ic = g["idxcol_base"]
        for k in range(ncls):
            k0, kn = g["cls_span"][k]
            if kn:
                ic = _emit_gather(msg, k0, kn, tabs[k][:, :], ic, elem)

    # ctx0: resident loads (own context so its exit barrier orders them
    # before every later consumer on every engine).
    with tile.TileContext(nc):
        nc.sync.dma_start(io_sb[:, :], io_d[:, :])
        nc.sync.dma_start(id_sb[:, :], id_d[:, :])
        nc.sync.dma_start(w1_sb[:, :, :], w1_d[:, :, :])
        nc.sync.dma_start(w2_sb[:, :, :], w2_d[:, :, :])
        nc.sync.dma_start(b1_sb[:, :], b1_d[:, :])
        nc.sync.dma_start(b2_sb[:, :], b2_d[:, :])
        nc.sync.dma_start(dv_sb[:, :], dv_d[:, :])
        nc.sync.dma_start(sq_sb[:, :], sq_d[:, :])

    MEGA = tile.TileContext(nc)
    tc_main = MEGA.__enter__()
    nc.sync.dma_start(idx_sb[:, :], ix_d[:, :])
    nc.sync.dma_start(dr_sb[:, :], dr_d[:, :])

    # layer-1 dense projection h1' = D^-1/2 (x @ W1), 2 tiles/step; the
    # class-k AllGather fires as soon as its producer tiles are written.
    def _dense1(tc, t_lo, t_hi):
        with (
            tc.tile_pool(name=f"pA{t_lo}", bufs=3) as pA,
            tc.tile_pool(name=f"psA{t_lo}", bufs=4, space="PSUM") as psA,
        ):
            for t0 in range(t_lo, t_hi, 2):
                nt = min(2, t_hi - t0)
                xa = pA.tile([P, nt, ncc, P], F16, tag="xa")
                nc.sync.dma_start(
                    xa[:, :, :, :],
                    xt_d[t0 : t0 + nt, :, :, :].rearrange("t p c n -> p t c n"),
                )
                for kk in range(nt):
                    ps = psA.tile([P, hid], F32, tag="psA")
                    for cc in range(ncc):
                        nc.tensor.matmul(
                            ps[:, :],
                            lhsT=xa[:, kk, cc, :],
                            rhs=w1_sb[:, cc, :],
                            start=(cc == 0),
                            stop=(cc == ncc - 1),
                        )
                    nc.scalar.activation(
                        h1loc[:, t0 + kk, :],
                        ps[:, :],
                        mybir.ActivationFunctionType.Copy,
                        scale=dv_sb[:, t0 + kk : t0 + kk + 1],
                    )
                nc.sync.dma_start(
                    h1_sh[t0 * P : (t0 + nt) * P, :].rearrange(
                        "(t p) f -> p t f", p=P
                    ),
                    h1loc[:, t0 : t0 + nt, :],
                )

    cls_hi = list(plan.cls_lo[1:]) + [tiles]
    for k in range(ncls):
        _dense1(tc_main, plan.cls_lo[k], cls_hi[k])
        _allgather(h1_sh[plan.cls_lo[k] * P : cls_hi[k] * P, :], h1_f[k][:, :])

    # layer-1 aggregate + relu + layer-2 dense projection, group by group;
    # the class-k h2 AllGather fires once its tiles are aggregated.
    def _agg1(tc, g_lo, g_hi):
        groups = plan.groups[g_lo:g_hi]
        with (
            tc.tile_pool(name=f"msgB{g_lo}", bufs=3) as msgB,
            tc.tile_pool(name=f"sB{g_lo}", bufs=3) as sB,
            tc.tile_pool(name=f"zB{g_lo}", bufs=3) as zB,
            tc.tile_pool(name=f"psB{g_lo}", bufs=4, space="PSUM") as psB,
            tc.tile_pool(name=f"psT{g_lo}", bufs=2, space="PSUM") as psT,
            tc.tile_pool(name=f"psH{g_lo}", bufs=2, space="PSUM") as psH,
        ):
            for g in groups:
                msg = msgB.tile([P, g["nch"], hid], F8, tag="msg")
                _emit_group_gathers(g, msg, h1_f, hid)
                for t in g["tiles"]:
                    spans = g["tile_chunks"][t]
                    nch = sum(kn for _, kn in spans)
                    # one-hot S, one DVE is_equal per class slice
                    S = sB.tile([P, nch, P], F8, tag="S")
                    soff = 0
                    for k0, kn in spans:
                        if not kn:
                            continue
                        nc.vector.tensor_tensor(
                            out=S[:, soff : soff + kn, :],
                            in0=io_sb[:, None, :].to_broadcast([P, kn, P]),
                            in1=dr_sb[
                                :, g["chunk_base"] + k0 : g["chunk_base"] + k0 + kn
                            ][:, :, None].to_broadcast([P, kn, P]),
                            op=mybir.AluOpType.is_equal,
                        )
                        soff += kn
                    ps = psB.tile([P, hid], F32, tag="psB")
                    # rank-1 bias + self-loop first: independent of gathers
                    nc.tensor.matmul(
                        ps[:, :],
                        lhsT=sq_sb[0:1, t * P : (t + 1) * P],
                        rhs=b1_sb[0:1, :],
                        start=True,
                        stop=False,
                    )
                    nc.tensor.matmul(
                        ps[:, :],
                        lhsT=id_sb[:, :],
                        rhs=h1loc[:, t, :],
                        start=False,
                        stop=(nch == 0),
                    )
                    j = 0
                    for k0, kn in spans:
                        for i in range(kn):
                            nc.tensor.matmul(
                                ps[:, :],
                                lhsT=S[:, j, :],
                                rhs=msg[:, k0 + i, :],
                                start=False,
                                stop=(j == nch - 1),
                            )
                            j += 1
                    # z1 = relu(dinv[n] * ps)  [node, f] fp16
                    z1 = zB.tile([P, hid], F16, tag="z1")
                    nc.vector.tensor_scalar(
                        out=z1[:, :],
                        in0=ps[:, :],
                        scalar1=dv_sb[:, t : t + 1],
                        scalar2=0.0,
                        op0=mybir.AluOpType.mult,
                        op1=mybir.AluOpType.max,
                    )
                    # transpose z1 -> z1T for the layer-2 contraction
                    zt_ps = psT.tile([P, hcc, P], F16, tag="ztps")
                    for h in range(hcc):
                        nc.tensor.transpose(
                            zt_ps[:, h, :],
                            z1[:, h * P : (h + 1) * P],
                            id_sb[:, :],
                        )
                    zt = zB.tile([P, hcc, P], F16, tag="zt")
                    nc.scalar.activation(
                        zt[:, :, :],
                        zt_ps[:, :, :],
                        mybir.ActivationFunctionType.Copy,
                    )
                    hps = psH.tile([P, outc], F32, tag="hps")
                    for cc in range(hcc):
                        nc.tensor.matmul(
                            hps[:, :],
                            lhsT=zt[:, cc, :],
                            rhs=w2_sb[:, cc, :],
                            start=(cc == 0),
                            stop=(cc == hcc - 1),
                        )
                    # h2' = dinv[n] * (z1 @ W2)
                    nc.scalar.activation(
                        h2loc[:, t, :],
                        hps[:, :],
                        mybir.ActivationFunctionType.Copy,
                        scale=dv_sb[:, t : t + 1],
                    )
                    nc.sync.dma_start(
                        h2_sh[t * P : (t + 1) * P, :], h2loc[:, t, :]
                    )

    for k in range(ncls):
        _agg1(tc_main, plan.cls_lo[k] // GRP, _cdiv(cls_hi[k], GRP))
        _allgather(h2_sh[plan.cls_lo[k] * P : cls_hi[k] * P, :], h2_f[k][:, :])

    # layer-2 aggregate + bias -> output
    with (
        tc_main.tile_pool(name="msgC", bufs=3) as msgC,
        tc_main.tile_pool(name="sC", bufs=3) as sC,
        tc_main.tile_pool(name="oC", bufs=3) as oC,
        tc_main.tile_pool(name="psC", bufs=6, space="PSUM") as psC,
    ):
        for g in plan.groups:
            msg = msgC.tile([P, g["nch"], outc], F16, tag="msg2")
            _emit_group_gathers(g, msg, h2_f, outc)
            for t in g["tiles"]:
                spans = g["tile_chunks"][t]
                nch = sum(kn for _, kn in spans)
                S = sC.tile([P, nch, P], F8, tag="S2")
                soff = 0
                for k0, kn in spans:
                    if not kn:
                        continue
                    nc.vector.tensor_tensor(
                        out=S[:, soff : soff + kn, :],
                        in0=io_sb[:, None, :].to_broadcast([P, kn, P]),
                        in1=dr_sb[
                            :, g["chunk_base"] + k0 : g["chunk_base"] + k0 + kn
                        ][:, :, None].to_broadcast([P, kn, P]),
                        op=mybir.AluOpType.is_equal,
                    )
                    soff += kn
                ps = psC.tile([P, outc], F32, tag="psC")
                nc.tensor.matmul(
                    ps[:, :],
                    lhsT=sq_sb[0:1, t * P : (t + 1) * P],
                    rhs=b2_sb[0:1, :],
                    start=True,
                    stop=False,
                )
                nc.tensor.matmul(
                    ps[:, :],
                    lhsT=id_sb[:, :],
                    rhs=h2loc[:, t, :],
                    start=False,
                    stop=(nch == 0),
                )
                j = 0
                for k0, kn in spans:
                    for i in range(kn):
                        nc.tensor.matmul(
                            ps[:, :],
                            lhsT=S[:, j, :],
                            rhs=msg[:, k0 + i, :],
                            start=False,
                            stop=(j == nch - 1),
                        )
                        j += 1
                ob = oC.tile([P, outc], F32, tag="ob")
                nc.vector.tensor_scalar(
                    out=ob[:, :],
                    in0=ps[:, :],
                    scalar1=dv_sb[:, t : t + 1],
                    scalar2=None,
                    op0=mybir.AluOpType.mult,
                )
                nc.sync.dma_start(out_d[t * P : (t + 1) * P, :], ob[:, :])

    MEGA.__exit__(None, None, None)
    st.close()
    nc.compile()
    return nc


def _make_in_maps(plan, x, W1, b1, W2, b2):
    inc, hid, outc = plan.inc, plan.hid, plan.outc
    ncc, hcc = inc // P, hid // P
    w1t = np.ascontiguousarray(
        W1.reshape(ncc, P, hid).transpose(1, 0, 2).astype(np.float16)
    )
    w2t = np.ascontiguousarray(
        W2.reshape(hcc, P, outc).transpose(1, 0, 2).astype(np.float16)
    )
    b1r = np.ascontiguousarray(b1.astype(np.float16).reshape(1, hid))
    b2r = np.ascontiguousarray(b2.astype(np.float16).reshape(1, outc))
    iota = np.ascontiguousarray(
        np.tile(np.arange(P, dtype=np.float32), (P, 1))
    )
    ident = np.ascontiguousarray(np.eye(P, dtype=np.float16))
    in_maps = []
    for c in range(CORES):
        xs = x[c * plan.shard : (c + 1) * plan.shard].astype(np.float32)
        xs = np.pad(xs, ((0, plan.shard_pad - plan.shard), (0, 0)))
        xt = xs.reshape(plan.tiles, P, ncc, P).transpose(0, 3, 2, 1)
        in_maps.append(
            {
                "xt": np.ascontiguousarray(xt.astype(np.float16)),
                "w1t": w1t,
                "w2t": w2t,
                "b1r": b1r,
                "b2r": b2r,
                "iota": iota,
                "ident": ident,
                "idx": plan.core_idx[c],
                "dstrel": plan.core_drel[c],
                "dinvc": plan.core_dinvc[c],
                "sqd": plan.core_sqd[c],
            }
        )
    return in_maps


_CACHE = {}


def _get_built(x, edge_index, W1, b1, W2, b2):
    n_nodes, in_c = x.shape
    hid = W1.shape[1]
    out_c = W2.shape[1]
    key = (n_nodes, in_c, hid, out_c, hash(edge_index.tobytes()))
    if key not in _CACHE:
        src = np.asarray(edge_index[0], np.int64)
        dst = np.asarray(edge_index[1], np.int64)
        plan = Plan(n_nodes, src, dst, in_c, hid, out_c)
        nc = _build_nc(plan)
        _CACHE[key] = (plan, nc)
    return _CACHE[key]


def run(x, edge_index, W1, b1, W2, b2, trace=False, **spmd_kwargs):
    plan, nc = _get_built(x, edge_index, W1, b1, W2, b2)
    in_maps = _make_in_maps(plan, x, W1, b1, W2, b2)
    res = run_bass_kernel_spmd(
        nc, in_maps, core_ids=list(range(CORES)), trace=trace, **spmd_kwargs
    )
    out = np.concatenate(
        [res.results[c]["out"][: plan.shard] for c in range(CORES)], axis=0
    ).astype(np.float32)
    return out, res


def kernel(**inputs):
    x = np.asarray(inputs["x"], np.float32)
    edge_index = np.asarray(inputs["edge_index"])
    W1 = np.asarray(inputs["W1"], np.float32)
    b1 = np.asarray(inputs["b1"], np.float32)
    W2 = np.asarray(inputs["W2"], np.float32)
    b2 = np.asarray(inputs["b2"], np.float32)
    out, _ = run(x, edge_index, W1, b1, W2, b2)
    return out
